# revision 30
# baseline (speedup 1.0000x reference)
"""HarmonicCausalSelfAttention on 8 TRN2 NeuronCores — v2.

Sharding: core c -> (batch b = c//2, head-group g = c%2); each core computes
attention for 8 heads of one batch and a full-width partial of the output
projection; the host sums the two partials per batch (the rank-128 c-proj
intermediate is linear, so out = (r_g0 + r_g1) @ cB^T = part_g0 + part_g1).

v2 changes vs v1 (608 us baseline):
  - heads processed in PAIRS stacked on partitions: Q^T/K^T tiles are
    [128 = 2 heads x 64d, T]; score matmuls for the two heads run
    CONCURRENTLY in disjoint 64-row groups of the PE array (tile_position
    auto-derived from partition offsets) -> ~2x score throughput.
  - QK production and c_proj stage-1 use full 128-wide stationaries
    (pair-packed) instead of 64-wide.
  - softmax denominators batched into a [4, 512] tile per (pair, hc-half)
    and inverted with ONE reciprocal_approx_fast (v1 spent 107 us on 32
    [1,512] full-precision reciprocals).
  - 1/denom broadcast along partitions via gpsimd partition_broadcast
    (idle engine) instead of a PE ones-matmul + extra DVE copy.
  - causal mask applied POST-exp as a bf16 0/1 multiply (2x DVE mode)
    instead of a -30000 f32 add on PSUM (1x mode).
  - PSUM->SBUF evacuations split between VectorE and ScalarE in the
    phases where ScalarE is not busy with exp.
  - output returned in bf16 (halves the output DMA; host sums in f32).

Layout strategy: everything transposed so no on-device transposes needed.
  stage1:  t^T[rank, T]    = A @ x^T           (x^T prepared host-side, bf16)
  stage2:  QT2/KT2[128, hp, T]  pair-stacked;  V[keys, 64+ones] keys-major
  attn:    S^T[keys, q] = K_kb @ Q^T   per head, 2 heads concurrent
           P^T = exp(S^T / 8) on ScalarE (PSUM -> SBUF bf16), 0/1 diag mask
           PV:  psum[65, 512q] += [V_kb | 1]^T @ P^T_kb  (row 64 = denom)
           normalize: batched recip -> gpsimd broadcast -> DVE multiply
  c_proj:  r^T[rank, T] = sum_hp cA_hp2 @ YT2_hp ; out = r^T chunks.T @ cB^T
"""

import numpy as np
import ml_dtypes

import concourse.bass as bass
from concourse import bacc
import concourse.mybir as mybir
from concourse.tile import TileContext
from concourse.bass_utils import run_bass_kernel_spmd

B, T, C = 4, 2048, 1024
NH, HD = 16, 64
RANK = 128
NCORES = 8
HPC = 8          # heads per core
NPAIR = 4        # head pairs per core
G = 512          # C columns per head group
P = 128
F32 = mybir.dt.float32
BF16 = mybir.dt.bfloat16
BF = ml_dtypes.bfloat16

_NC_CACHE = None


def _chunks(total, step):
    res = []
    o = 0
    while o < total:
        res.append((o, min(step, total - o)))
        o += min(step, total - o)
    return res


def build():
    nc = bacc.Bacc()
    dp = nc.declare_dram_parameter
    xT = dp("xT", [C, T], BF16, isOutput=False)
    qAT = dp("qAT", [C, RANK], BF16, isOutput=False)
    kAT = dp("kAT", [C, RANK], BF16, isOutput=False)
    vAT = dp("vAT", [C, RANK], BF16, isOutput=False)
    qBT = dp("qBT", [RANK, G], BF16, isOutput=False)
    kBT = dp("kBT", [RANK, G], BF16, isOutput=False)
    vBT = dp("vBT", [RANK, G], BF16, isOutput=False)
    cAT = dp("cAT", [G, RANK], BF16, isOutput=False)
    cBT = dp("cBT", [RANK, C], BF16, isOutput=False)
    maskp = dp("mask01", [P, P], BF16, isOutput=False)
    out = dp("out", [T, C], BF16, isOutput=True)

    Exp = mybir.ActivationFunctionType.Exp
    MUL = mybir.AluOpType.mult

    with TileContext(nc) as tc:
        with tc.tile_pool(name="sb", bufs=1) as sb:
            vAT_sb0 = sb.tile([P, 8, RANK], BF16, tag="vAT")
            nc.gpsimd.dma_start(out=vAT_sb0, in_=vAT.rearrange("(co ci) r -> ci co r", ci=P))
            xT_sb = sb.tile([P, 8, T], BF16, tag="xT")
            xTr = xT.rearrange("(co ci) t -> ci co t", ci=P)
            for cc in range(8):
                eng = nc.gpsimd if cc % 2 == 0 else nc.sync
                eng.dma_start(out=xT_sb[:, cc, :], in_=xTr[:, cc, :])
            qAT_sb = sb.tile([P, 8, RANK], BF16, tag="qAT")
            nc.gpsimd.dma_start(out=qAT_sb, in_=qAT.rearrange("(co ci) r -> ci co r", ci=P))
            kAT_sb = sb.tile([P, 8, RANK], BF16, tag="kAT")
            nc.gpsimd.dma_start(out=kAT_sb, in_=kAT.rearrange("(co ci) r -> ci co r", ci=P))
            vAT_sb = vAT_sb0
            qBT_sb = sb.tile([RANK, G], BF16, tag="qBT")
            nc.gpsimd.dma_start(out=qBT_sb, in_=qBT[:, :])
            kBT_sb = sb.tile([RANK, G], BF16, tag="kBT")
            nc.gpsimd.dma_start(out=kBT_sb, in_=kBT[:, :])
            vBT_sb = sb.tile([RANK, G], BF16, tag="vBT")
            nc.gpsimd.dma_start(out=vBT_sb, in_=vBT[:, :])
            cAT2_sb = sb.tile([P, NPAIR, RANK], BF16, tag="cAT")
            nc.gpsimd.dma_start(out=cAT2_sb, in_=cAT.rearrange("(hp p) r -> p hp r", p=P))
            cBT_sb = sb.tile([RANK, C], BF16, tag="cBT")
            nc.gpsimd.dma_start(out=cBT_sb, in_=cBT[:, :])
            mask_sb = sb.tile([P, P], BF16, tag="mask")
            nc.gpsimd.dma_start(out=mask_sb, in_=maskp[:, :])

            QT2 = sb.tile([P, NPAIR, T], BF16, tag="QT2")
            KT2 = sb.tile([P, NPAIR, T], BF16, tag="KT2")
            YT2 = sb.tile([P, NPAIR, T], BF16, tag="YT2")
            V_sb = sb.tile([P, 16, HPC, P], BF16, tag="Vsb")
            tTq = sb.tile([P, T], BF16, tag="tTq")
            tTk = sb.tile([P, T], BF16, tag="tTk")
            tTv = sb.tile([P, T], BF16, tag="tTv")
            rT_sb = sb.tile([P, T], BF16, tag="rT")

            nc.gpsimd.memset(V_sb[:, :, :, 64:P], 1.0)
            rstg = sb.tile([P, 512], F32, tag="rstg")
            nc.gpsimd.memset(rstg, 1.0)
            rcp_sb = sb.tile([P, 512], F32, tag="rcp")

            # ---- phase A: t^T = A @ x^T for q,k,v ----
            with (
                tc.tile_pool(name="psA", bufs=2, space="PSUM") as psA,
                tc.tile_pool(name="psB", bufs=2, space="PSUM") as psB,
                tc.tile_pool(name="psV", bufs=2, space="PSUM") as psV,
            ):
                for pi, (AT_sb, tT) in enumerate(
                    ((vAT_sb, tTv), (qAT_sb, tTq), (kAT_sb, tTk))
                ):
                    for th in range(2):
                        h0 = th * 1024
                        pt = psA.tile([P, 1024], F32, tag="psA",
                                      name=f"psA{pi}_{th}")
                        for cc in range(8):
                            for t0, tw in _chunks(1024, 512):
                                nc.tensor.matmul(
                                    pt[:, t0:t0 + tw],
                                    AT_sb[:, cc, :],
                                    xT_sb[:, cc, h0 + t0:h0 + t0 + tw],
                                    start=(cc == 0), stop=(cc == 7),
                                )
                        nc.vector.tensor_copy(out=tT[:, h0:h0 + 1024], in_=pt)

                # ---- phase B: V keys-major with ones column ----
                for ti in range(16):
                    pv = psV.tile([P, G], F32, tag="psV")
                    nc.tensor.matmul(
                        pv, tTv[:, ti * 128:(ti + 1) * 128], vBT_sb,
                        start=True, stop=True,
                    )
                    nc.vector.tensor_copy(
                        out=V_sb[:, ti, :, 0:64],
                        in_=pv.rearrange("p (h d) -> p h d", d=64),
                    )

                # ---- phase B: pair-stacked Q^T, K^T  (M=128 = 2 heads) ----
                for BT_sb, dest, tT in ((qBT_sb, QT2, tTq), (kBT_sb, KT2, tTk)):
                    for hp in range(NPAIR):
                        for t0, tw in _chunks(T, 512):
                            p2 = psB.tile([P, 512], F32, tag="psB")
                            nc.tensor.matmul(
                                p2[:, :tw],
                                BT_sb[:, hp * P:(hp + 1) * P],
                                tT[:, t0:t0 + tw],
                                start=True, stop=True,
                            )
                            nc.vector.tensor_copy(
                                out=dest[:, hp, t0:t0 + tw], in_=p2[:, :tw])

            # ---- attention: head pairs, 2 concurrent row-groups ----
            # Strip loop is software-pipelined: strip kb+1's score matmuls are
            # emitted BEFORE strip kb's PV matmuls so the PE FIFO always has
            # exp-independent work while ScalarE runs exp on the prior strip.
            with (
                tc.tile_pool(name="psS", bufs=2, space="PSUM") as psS,
                tc.tile_pool(name="psPV", bufs=4, space="PSUM") as psPV,
                tc.tile_pool(name="ptp", bufs=6) as ptp,
                tc.tile_pool(name="nrm", bufs=8) as nrm,
            ):
                for hp in range(NPAIR):
                    for hc in range(2):
                        # pvt[e][j2]: accumulators for head e of the pair,
                        # q-panel j = 2*hc + j2; row 64 = softmax denominator
                        pvt = [[psPV.tile([P, 512], F32, tag="pv",
                                          name=f"pv{hp}_{hc}_{e}_{j2}")
                                for j2 in range(2)] for e in range(2)]
                        nkb = 8 if hc == 0 else 16

                        def emit_scores(kb, e, sps):
                            qlo = max(1024 * hc, 128 * kb)
                            w = 1024 * (hc + 1) - qlo
                            for c0, cw in _chunks(w, 512):
                                nc.tensor.matmul(
                                    sps[:, c0:c0 + cw],
                                    KT2[64 * e:64 * e + 64, hp,
                                        kb * 128:(kb + 1) * 128],
                                    QT2[64 * e:64 * e + 64, hp,
                                        qlo + c0:qlo + c0 + cw],
                                    start=True, stop=True,
                                )

                        def emit_exp(kb, e, sps, ptile):
                            qlo = max(1024 * hc, 128 * kb)
                            w = 1024 * (hc + 1) - qlo
                            nc.scalar.activation(
                                ptile[:, :w], sps[:, :w], Exp, scale=0.125)
                            if kb // 8 == hc:
                                nc.vector.tensor_tensor(
                                    out=ptile[:, 0:P], in0=ptile[:, 0:P],
                                    in1=mask_sb, op=MUL)

                        def emit_pv(kb, e, ptile):
                            qlo = max(1024 * hc, 128 * kb)
                            for j2 in range(2):
                                j = 2 * hc + j2
                                r0 = 512 * j
                                if 128 * kb >= r0 + 512:
                                    continue
                                c0 = max(qlo, r0)
                                cw = r0 + 512 - c0
                                nc.tensor.matmul(
                                    pvt[e][j2][:, c0 - r0:c0 - r0 + cw],
                                    V_sb[:, kb, 2 * hp + e, :],
                                    ptile[:, c0 - qlo:c0 - qlo + cw],
                                    start=(kb == 0), stop=(kb == 4 * j + 3),
                                )
                            for j2 in range(2):
                                j = 2 * hc + j2
                                if kb == 4 * j + 3:
                                    r0 = 512 * j
                                    nc.vector.tensor_copy(
                                        out=rstg[0:64, :],
                                        in_=pvt[e][j2][64:P, :])
                                    nc.vector.reciprocal_approx_fast(
                                        out=rcp_sb, in_=rstg)
                                    nc.vector.tensor_tensor(
                                        out=YT2[64 * e:64 * e + 64, hp,
                                                r0:r0 + 512],
                                        in0=pvt[e][j2][0:64, :],
                                        in1=rcp_sb[0:64, :], op=MUL)

                        # software pipeline, interleaved per head lane so every
                        # PE instruction is ready when the FIFO reaches it:
                        #   sc_e(kb) | pv_e(kb-1) | sc_o(kb) | pv_o(kb-1)
                        prev = None
                        for kb in range(nkb):
                            sps = [psS.tile([P, 1024], F32, tag="s",
                                            name=f"s{hp}_{hc}_{kb}_{e}")
                                   for e in range(2)]
                            ptile = [ptp.tile([P, 1024], BF16, tag="pt",
                                              name=f"p{hp}_{hc}_{kb}_{e}")
                                     for e in range(2)]
                            for e in range(2):
                                emit_scores(kb, e, sps[e])
                                emit_exp(kb, e, sps[e], ptile[e])
                                if prev is not None:
                                    emit_pv(prev[0], e, prev[1][e])
                            prev = (kb, ptile)
                        for e in range(2):
                            emit_pv(prev[0], e, prev[1][e])

            # ---- phase D: c_proj ----
            with (
                tc.tile_pool(name="psD", bufs=1, space="PSUM") as psD,
                tc.tile_pool(name="psO", bufs=3, space="PSUM") as psO,
                tc.tile_pool(name="ost", bufs=3) as ost,
            ):
                pr = psD.tile([P, T], F32, tag="r")
                for hp in range(NPAIR):
                    for t0, tw in _chunks(T, 512):
                        nc.tensor.matmul(
                            pr[:, t0:t0 + tw], cAT2_sb[:, hp, :],
                            YT2[:, hp, t0:t0 + tw],
                            start=(hp == 0), stop=(hp == NPAIR - 1),
                        )
                for t0, tw in _chunks(T, 512):
                    nc.vector.tensor_copy(
                        out=rT_sb[:, t0:t0 + tw], in_=pr[:, t0:t0 + tw])
                for ti in range(16):
                    for nn in range(2):
                        po = psO.tile([P, 512], F32, tag="o")
                        nc.tensor.matmul(
                            po, rT_sb[:, ti * 128:(ti + 1) * 128],
                            cBT_sb[:, nn * 512:(nn + 1) * 512],
                            start=True, stop=True,
                        )
                        ob = ost.tile([P, 512], BF16, tag="ob")
                        if (2 * ti + nn) % 2 == 0:
                            nc.vector.tensor_copy(out=ob, in_=po)
                        else:
                            nc.scalar.copy(out=ob, in_=po)
                        deng = nc.sync if (2 * ti + nn) % 2 == 0 else nc.gpsimd
                        deng.dma_start(
                            out=out[ti * 128:(ti + 1) * 128,
                                    nn * 512:(nn + 1) * 512],
                            in_=ob,
                        )
    nc.finalize()
    return nc


def make_in_maps(x, qA, qB, kA, kB, vA, vB, cA, cB):
    x, qA, qB, kA, kB, vA, vB, cA, cB = [
        np.asarray(a, dtype=np.float32) for a in (x, qA, qB, kA, kB, vA, vB, cA, cB)
    ]
    mask01 = np.where(
        np.arange(P)[:, None] <= np.arange(P)[None, :], 1.0, 0.0
    ).astype(BF)
    qATn = np.ascontiguousarray(qA.T).astype(BF)
    kATn = np.ascontiguousarray(kA.T).astype(BF)
    vATn = np.ascontiguousarray(vA.T).astype(BF)
    cBTn = np.ascontiguousarray(cB.T).astype(BF)
    in_maps = []
    for c in range(NCORES):
        b, g = divmod(c, 2)
        sl = slice(g * G, (g + 1) * G)
        in_maps.append({
            "xT": np.ascontiguousarray(x[b].T).astype(BF),
            "qAT": qATn, "kAT": kATn, "vAT": vATn,
            "qBT": np.ascontiguousarray(qB[sl, :].T).astype(BF),
            "kBT": np.ascontiguousarray(kB[sl, :].T).astype(BF),
            "vBT": np.ascontiguousarray(vB[sl, :].T).astype(BF),
            "cAT": np.ascontiguousarray(cA[:, sl].T).astype(BF),
            "cBT": cBTn,
            "mask01": mask01,
        })
    return in_maps


def combine(parts):
    return np.stack(
        [parts[2 * b].astype(np.float32) + parts[2 * b + 1].astype(np.float32)
         for b in range(B)], axis=0)


def kernel(x, qA, qB, kA, kB, vA, vB, cA, cB):
    global _NC_CACHE
    if _NC_CACHE is None:
        _NC_CACHE = build()
    in_maps = make_in_maps(x, qA, qB, kA, kB, vA, vB, cA, cB)
    res = run_bass_kernel_spmd(_NC_CACHE, in_maps, list(range(NCORES))).results
    return combine([res[c]["out"] for c in range(NCORES)])


# revision 32
# speedup vs baseline: 1.0241x; 1.0241x over previous
"""HarmonicCausalSelfAttention on 8 TRN2 NeuronCores.

Sharding: core c -> (batch b = c//2, head-group g = c%2); each core computes
attention for 8 heads of one batch and a full-width partial of the output
projection; the host sums the two partials per batch (the rank-128 c-proj
intermediate is linear, so out = (r_g0 + r_g1) @ cB^T = part_g0 + part_g1).

Layout: everything transposed so no on-device transposes are needed.
  stage1:  t^T[rank, T] = A @ x^T       (x^T prepared host-side, bf16;
           xT streamed in per-128-row slice so stage1 starts early)
  stage2:  QT2/KT2[128 = 2 heads x 64d, hp, T] pair-stacked (full 128-wide
           stationaries for production); V keys-major [keys, 128] where
           cols 64:128 are ALL ONES.
  attn per head pair, per q-half (hc), per 128-key strip kb:
           S^T[keys, q] = K_kb @ Q^T  (half-array K=64, partition-offset 64
           selects the odd head's rows)
           P^T = exp(S^T / 8) on ScalarE (PSUM -> SBUF bf16); causal diag
           masked POST-exp by a bf16 0/1 multiply (2x DVE mode).
           PV: psum[128, 512q] += V_kb^T @ P^T_kb — rows 64:128 of V are
           ones, so psum rows 64:128 hold 64 REPLICATED copies of the
           softmax denominator: normalization needs no partition broadcast:
             stage rows 64:128 -> SBUF, reciprocal_approx_fast [128,512]
             (one custom-DVE op), multiply rows 0:64 by it -> Y^T bf16.
           The strip loop is software-pipelined and interleaved per head
           lane (sc_e(kb) | pv_e(kb-1) | sc_o(kb) | pv_o(kb-1)) so every PE
           instruction's semaphore is satisfied when the FIFO reaches it.
  c_proj:  r^T[rank, T] = sum_hp cA_hp2[128d x rank] @ YT2_hp (pair-packed
           K=128); out chunks = r^T.T @ cB^T, cast bf16, DMA out.
ScalarE runs ONLY exp (PSUM-source activations); all other PSUM
evacuations are VectorE (keeping ScalarE quiet measurably reduces PE
throttling). Output returned in bf16; host sums partials in f32.

Measured on 8-core trn2 (neuron-profile): 608 us (session-start baseline)
-> 338 us, rel err 7.8e-3.
"""

import numpy as np
import ml_dtypes

import concourse.bass as bass
from concourse import bacc
import concourse.mybir as mybir
from concourse.tile import TileContext
from concourse.bass_utils import run_bass_kernel_spmd

B, T, C = 4, 2048, 1024
NH, HD = 16, 64
RANK = 128
NCORES = 8
HPC = 8          # heads per core
NPAIR = 4        # head pairs per core
G = 512          # C columns per head group
P = 128
F32 = mybir.dt.float32
BF16 = mybir.dt.bfloat16
BF = ml_dtypes.bfloat16

_NC_CACHE = None


def _chunks(total, step):
    res = []
    o = 0
    while o < total:
        res.append((o, min(step, total - o)))
        o += min(step, total - o)
    return res


def build():
    nc = bacc.Bacc()
    dp = nc.declare_dram_parameter
    xT = dp("xT", [C, T], BF16, isOutput=False)
    qAT = dp("qAT", [C, RANK], BF16, isOutput=False)
    kAT = dp("kAT", [C, RANK], BF16, isOutput=False)
    vAT = dp("vAT", [C, RANK], BF16, isOutput=False)
    qBT = dp("qBT", [RANK, G], BF16, isOutput=False)
    kBT = dp("kBT", [RANK, G], BF16, isOutput=False)
    vBT = dp("vBT", [RANK, G], BF16, isOutput=False)
    cAT = dp("cAT", [G, RANK], BF16, isOutput=False)
    cBT = dp("cBT", [RANK, C], BF16, isOutput=False)
    maskp = dp("mask01", [P, P], BF16, isOutput=False)
    out = dp("out", [T, C], BF16, isOutput=True)

    Exp = mybir.ActivationFunctionType.Exp
    MUL = mybir.AluOpType.mult

    with TileContext(nc) as tc:
        with tc.tile_pool(name="sb", bufs=1) as sb:
            vAT_sb0 = sb.tile([P, 8, RANK], BF16, tag="vAT")
            nc.gpsimd.dma_start(out=vAT_sb0, in_=vAT.rearrange("(co ci) r -> ci co r", ci=P))
            xT_sb = sb.tile([P, 8, T], BF16, tag="xT")
            xTr = xT.rearrange("(co ci) t -> ci co t", ci=P)
            for cc in range(8):
                nc.gpsimd.dma_start(out=xT_sb[:, cc, :], in_=xTr[:, cc, :])
            qAT_sb = sb.tile([P, 8, RANK], BF16, tag="qAT")
            nc.gpsimd.dma_start(out=qAT_sb, in_=qAT.rearrange("(co ci) r -> ci co r", ci=P))
            kAT_sb = sb.tile([P, 8, RANK], BF16, tag="kAT")
            nc.gpsimd.dma_start(out=kAT_sb, in_=kAT.rearrange("(co ci) r -> ci co r", ci=P))
            vAT_sb = vAT_sb0
            qBT_sb = sb.tile([RANK, G], BF16, tag="qBT")
            nc.gpsimd.dma_start(out=qBT_sb, in_=qBT[:, :])
            kBT_sb = sb.tile([RANK, G], BF16, tag="kBT")
            nc.gpsimd.dma_start(out=kBT_sb, in_=kBT[:, :])
            vBT_sb = sb.tile([RANK, G], BF16, tag="vBT")
            nc.gpsimd.dma_start(out=vBT_sb, in_=vBT[:, :])
            cAT2_sb = sb.tile([P, NPAIR, RANK], BF16, tag="cAT")
            nc.gpsimd.dma_start(out=cAT2_sb, in_=cAT.rearrange("(hp p) r -> p hp r", p=P))
            cBT_sb = sb.tile([RANK, C], BF16, tag="cBT")
            nc.gpsimd.dma_start(out=cBT_sb, in_=cBT[:, :])
            mask_sb = sb.tile([P, P], BF16, tag="mask")
            nc.gpsimd.dma_start(out=mask_sb, in_=maskp[:, :])

            QT2 = sb.tile([P, NPAIR, T], BF16, tag="QT2")
            KT2 = sb.tile([P, NPAIR, T], BF16, tag="KT2")
            YT2 = sb.tile([P, NPAIR, T], BF16, tag="YT2")
            V_sb = sb.tile([P, 16, HPC, P], BF16, tag="Vsb")
            tTq = sb.tile([P, T], BF16, tag="tTq")
            tTk = sb.tile([P, T], BF16, tag="tTk")
            tTv = sb.tile([P, T], BF16, tag="tTv")
            rT_sb = sb.tile([P, T], BF16, tag="rT")

            nc.gpsimd.memset(V_sb[:, :, :, 64:P], 1.0)
            rstg = sb.tile([P, 512], F32, tag="rstg")
            nc.gpsimd.memset(rstg, 1.0)
            rcp_sb = sb.tile([P, 512], F32, tag="rcp")

            # ---- phase A: t^T = A @ x^T for q,k,v ----
            with (
                tc.tile_pool(name="psA", bufs=2, space="PSUM") as psA,
                tc.tile_pool(name="psB", bufs=2, space="PSUM") as psB,
                tc.tile_pool(name="psV", bufs=2, space="PSUM") as psV,
            ):
                for pi, (AT_sb, tT) in enumerate(
                    ((vAT_sb, tTv), (qAT_sb, tTq), (kAT_sb, tTk))
                ):
                    for th in range(2):
                        h0 = th * 1024
                        pt = psA.tile([P, 1024], F32, tag="psA",
                                      name=f"psA{pi}_{th}")
                        for cc in range(8):
                            for t0, tw in _chunks(1024, 512):
                                nc.tensor.matmul(
                                    pt[:, t0:t0 + tw],
                                    AT_sb[:, cc, :],
                                    xT_sb[:, cc, h0 + t0:h0 + t0 + tw],
                                    start=(cc == 0), stop=(cc == 7),
                                )
                        nc.vector.tensor_copy(out=tT[:, h0:h0 + 1024], in_=pt)

                # ---- phase B: V keys-major with ones column ----
                for ti in range(16):
                    pv = psV.tile([P, G], F32, tag="psV")
                    nc.tensor.matmul(
                        pv, tTv[:, ti * 128:(ti + 1) * 128], vBT_sb,
                        start=True, stop=True,
                    )
                    nc.vector.tensor_copy(
                        out=V_sb[:, ti, :, 0:64],
                        in_=pv.rearrange("p (h d) -> p h d", d=64),
                    )

                # ---- phase B: pair-stacked Q^T, K^T  (M=128 = 2 heads) ----
                for BT_sb, dest, tT in ((qBT_sb, QT2, tTq), (kBT_sb, KT2, tTk)):
                    for hp in range(NPAIR):
                        for t0, tw in _chunks(T, 512):
                            p2 = psB.tile([P, 512], F32, tag="psB")
                            nc.tensor.matmul(
                                p2[:, :tw],
                                BT_sb[:, hp * P:(hp + 1) * P],
                                tT[:, t0:t0 + tw],
                                start=True, stop=True,
                            )
                            nc.vector.tensor_copy(
                                out=dest[:, hp, t0:t0 + tw], in_=p2[:, :tw])

            # ---- attention: head pairs, 2 concurrent row-groups ----
            # Strip loop is software-pipelined: strip kb+1's score matmuls are
            # emitted BEFORE strip kb's PV matmuls so the PE FIFO always has
            # exp-independent work while ScalarE runs exp on the prior strip.
            with (
                tc.tile_pool(name="psS", bufs=2, space="PSUM") as psS,
                tc.tile_pool(name="psPV", bufs=4, space="PSUM") as psPV,
                tc.tile_pool(name="ptp", bufs=6) as ptp,
                tc.tile_pool(name="nrm", bufs=8) as nrm,
            ):
                for hp in range(NPAIR):
                    for hc in range(2):
                        # pvt[e][j2]: accumulators for head e of the pair,
                        # q-panel j = 2*hc + j2; row 64 = softmax denominator
                        pvt = [[psPV.tile([P, 512], F32, tag="pv",
                                          name=f"pv{hp}_{hc}_{e}_{j2}")
                                for j2 in range(2)] for e in range(2)]
                        nkb = 8 if hc == 0 else 16

                        def emit_scores(kb, e, sps):
                            qlo = max(1024 * hc, 128 * kb)
                            w = 1024 * (hc + 1) - qlo
                            for c0, cw in _chunks(w, 512):
                                nc.tensor.matmul(
                                    sps[:, c0:c0 + cw],
                                    KT2[64 * e:64 * e + 64, hp,
                                        kb * 128:(kb + 1) * 128],
                                    QT2[64 * e:64 * e + 64, hp,
                                        qlo + c0:qlo + c0 + cw],
                                    start=True, stop=True,
                                )

                        def emit_exp(kb, e, sps, ptile):
                            qlo = max(1024 * hc, 128 * kb)
                            w = 1024 * (hc + 1) - qlo
                            nc.scalar.activation(
                                ptile[:, :w], sps[:, :w], Exp, scale=0.125)
                            if kb // 8 == hc:
                                nc.vector.tensor_tensor(
                                    out=ptile[:, 0:P], in0=ptile[:, 0:P],
                                    in1=mask_sb, op=MUL)

                        def emit_pv(kb, e, ptile):
                            qlo = max(1024 * hc, 128 * kb)
                            for j2 in range(2):
                                j = 2 * hc + j2
                                r0 = 512 * j
                                if 128 * kb >= r0 + 512:
                                    continue
                                c0 = max(qlo, r0)
                                cw = r0 + 512 - c0
                                nc.tensor.matmul(
                                    pvt[e][j2][:, c0 - r0:c0 - r0 + cw],
                                    V_sb[:, kb, 2 * hp + e, :],
                                    ptile[:, c0 - qlo:c0 - qlo + cw],
                                    start=(kb == 0), stop=(kb == 4 * j + 3),
                                )
                            for j2 in range(2):
                                j = 2 * hc + j2
                                if kb == 4 * j + 3:
                                    r0 = 512 * j
                                    nc.vector.tensor_copy(
                                        out=rstg[0:64, :],
                                        in_=pvt[e][j2][64:P, :])
                                    nc.vector.reciprocal_approx_fast(
                                        out=rcp_sb, in_=rstg)
                                    nc.vector.tensor_tensor(
                                        out=YT2[64 * e:64 * e + 64, hp,
                                                r0:r0 + 512],
                                        in0=pvt[e][j2][0:64, :],
                                        in1=rcp_sb[0:64, :], op=MUL)

                        # software pipeline, interleaved per head lane so every
                        # PE instruction is ready when the FIFO reaches it:
                        #   sc_e(kb) | pv_e(kb-1) | sc_o(kb) | pv_o(kb-1)
                        prev = None
                        for kb in range(nkb):
                            sps = [psS.tile([P, 1024], F32, tag="s",
                                            name=f"s{hp}_{hc}_{kb}_{e}")
                                   for e in range(2)]
                            ptile = [ptp.tile([P, 1024], BF16, tag="pt",
                                              name=f"p{hp}_{hc}_{kb}_{e}")
                                     for e in range(2)]
                            for e in range(2):
                                emit_scores(kb, e, sps[e])
                                emit_exp(kb, e, sps[e], ptile[e])
                                if prev is not None:
                                    emit_pv(prev[0], e, prev[1][e])
                            prev = (kb, ptile)
                        for e in range(2):
                            emit_pv(prev[0], e, prev[1][e])

            # ---- phase D: c_proj ----
            with (
                tc.tile_pool(name="psD", bufs=1, space="PSUM") as psD,
                tc.tile_pool(name="psO", bufs=3, space="PSUM") as psO,
                tc.tile_pool(name="ost", bufs=3) as ost,
            ):
                pr = psD.tile([P, T], F32, tag="r")
                for hp in range(NPAIR):
                    for t0, tw in _chunks(T, 512):
                        nc.tensor.matmul(
                            pr[:, t0:t0 + tw], cAT2_sb[:, hp, :],
                            YT2[:, hp, t0:t0 + tw],
                            start=(hp == 0), stop=(hp == NPAIR - 1),
                        )
                for t0, tw in _chunks(T, 512):
                    nc.vector.tensor_copy(
                        out=rT_sb[:, t0:t0 + tw], in_=pr[:, t0:t0 + tw])
                for ti in range(16):
                    for nn in range(2):
                        po = psO.tile([P, 512], F32, tag="o")
                        nc.tensor.matmul(
                            po, rT_sb[:, ti * 128:(ti + 1) * 128],
                            cBT_sb[:, nn * 512:(nn + 1) * 512],
                            start=True, stop=True,
                        )
                        ob = ost.tile([P, 512], BF16, tag="ob")
                        nc.vector.tensor_copy(out=ob, in_=po)
                        nc.sync.dma_start(
                            out=out[ti * 128:(ti + 1) * 128,
                                    nn * 512:(nn + 1) * 512],
                            in_=ob,
                        )
    nc.finalize()
    return nc


def make_in_maps(x, qA, qB, kA, kB, vA, vB, cA, cB):
    x, qA, qB, kA, kB, vA, vB, cA, cB = [
        np.asarray(a, dtype=np.float32) for a in (x, qA, qB, kA, kB, vA, vB, cA, cB)
    ]
    mask01 = np.where(
        np.arange(P)[:, None] <= np.arange(P)[None, :], 1.0, 0.0
    ).astype(BF)
    qATn = np.ascontiguousarray(qA.T).astype(BF)
    kATn = np.ascontiguousarray(kA.T).astype(BF)
    vATn = np.ascontiguousarray(vA.T).astype(BF)
    cBTn = np.ascontiguousarray(cB.T).astype(BF)
    in_maps = []
    for c in range(NCORES):
        b, g = divmod(c, 2)
        sl = slice(g * G, (g + 1) * G)
        in_maps.append({
            "xT": np.ascontiguousarray(x[b].T).astype(BF),
            "qAT": qATn, "kAT": kATn, "vAT": vATn,
            "qBT": np.ascontiguousarray(qB[sl, :].T).astype(BF),
            "kBT": np.ascontiguousarray(kB[sl, :].T).astype(BF),
            "vBT": np.ascontiguousarray(vB[sl, :].T).astype(BF),
            "cAT": np.ascontiguousarray(cA[:, sl].T).astype(BF),
            "cBT": cBTn,
            "mask01": mask01,
        })
    return in_maps


def combine(parts):
    return np.stack(
        [parts[2 * b].astype(np.float32) + parts[2 * b + 1].astype(np.float32)
         for b in range(B)], axis=0)


def kernel(x, qA, qB, kA, kB, vA, vB, cA, cB):
    global _NC_CACHE
    if _NC_CACHE is None:
        _NC_CACHE = build()
    in_maps = make_in_maps(x, qA, qB, kA, kB, vA, vB, cA, cB)
    res = run_bass_kernel_spmd(_NC_CACHE, in_maps, list(range(NCORES))).results
    return combine([res[c]["out"] for c in range(NCORES)])


# revision 33
# speedup vs baseline: 1.1051x; 1.0790x over previous
"""HarmonicCausalSelfAttention on 8 TRN2 NeuronCores.

Sharding: core c -> (batch b = c//2, head-group g = c%2); each core computes
attention for 8 heads of one batch and a full-width partial of the output
projection; the host sums the two partials per batch (the rank-128 c-proj
intermediate is linear, so out = (r_g0 + r_g1) @ cB^T = part_g0 + part_g1).

Layout: everything transposed so no on-device transposes are needed.
  stage1:  t^T[rank, T] = A @ x^T       (x^T prepared host-side, bf16;
           xT streamed in per-128-row slice so stage1 starts early)
  stage2:  QT2/KT2[128 = 2 heads x 64d, hp, T] pair-stacked (full 128-wide
           stationaries for production); V keys-major [keys, 128] where
           cols 64:128 are ALL ONES.
  attn per head pair, per q-half (hc), per 128-key strip kb:
           S^T[keys, q] = K_kb @ Q^T  (half-array K=64, partition-offset 64
           selects the odd head's rows)
           P^T = exp(S^T / 8) on ScalarE (PSUM -> SBUF bf16); causal diag
           masked POST-exp by a bf16 0/1 multiply (2x DVE mode).
           PV: psum[128, 512q] += V_kb^T @ P^T_kb — rows 64:128 of V are
           ones, so psum rows 64:128 hold 64 REPLICATED copies of the
           softmax denominator: normalization needs no partition broadcast:
             stage rows 64:128 -> SBUF, reciprocal_approx_fast [128,512]
             (one custom-DVE op), multiply rows 0:64 by it -> Y^T bf16.
           The strip loop is software-pipelined and interleaved per head
           lane (sc_e(kb) | pv_e(kb-1) | sc_o(kb) | pv_o(kb-1)) so every PE
           instruction's semaphore is satisfied when the FIFO reaches it.
  c_proj:  r^T[rank, T] = sum_hp cA_hp2[128d x rank] @ YT2_hp (pair-packed
           K=128); out chunks = r^T.T @ cB^T, cast bf16, DMA out.
ScalarE runs ONLY exp (PSUM-source activations); all other PSUM
evacuations are VectorE (keeping ScalarE quiet measurably reduces PE
throttling). Output returned in bf16; host sums partials in f32.

Measured on 8-core trn2 (neuron-profile): 608 us (session-start baseline)
-> 338 us, rel err 7.8e-3.
"""

import numpy as np
import ml_dtypes

import concourse.bass as bass
from concourse import bacc
import concourse.mybir as mybir
from concourse.tile import TileContext
from concourse.bass_utils import run_bass_kernel_spmd

B, T, C = 4, 2048, 1024
NH, HD = 16, 64
RANK = 128
NCORES = 8
HPC = 8          # heads per core
NPAIR = 4        # head pairs per core
G = 512          # C columns per head group
P = 128
F32 = mybir.dt.float32
BF16 = mybir.dt.bfloat16
BF = ml_dtypes.bfloat16

_NC_CACHE = None


def _chunks(total, step):
    res = []
    o = 0
    while o < total:
        res.append((o, min(step, total - o)))
        o += min(step, total - o)
    return res


def build():
    nc = bacc.Bacc()
    dp = nc.declare_dram_parameter
    xT = dp("xT", [C, T], BF16, isOutput=False)
    qAT = dp("qAT", [C, RANK], BF16, isOutput=False)
    kAT = dp("kAT", [C, RANK], BF16, isOutput=False)
    vAT = dp("vAT", [C, RANK], BF16, isOutput=False)
    qBT = dp("qBT", [RANK, G], BF16, isOutput=False)
    kBT = dp("kBT", [RANK, G], BF16, isOutput=False)
    vBT = dp("vBT", [RANK, G], BF16, isOutput=False)
    cAT = dp("cAT", [G, RANK], BF16, isOutput=False)
    cBT = dp("cBT", [RANK, C], BF16, isOutput=False)
    maskp = dp("mask01", [P, P], BF16, isOutput=False)
    out = dp("out", [T, C], BF16, isOutput=True)

    Exp = mybir.ActivationFunctionType.Exp
    MUL = mybir.AluOpType.mult

    with TileContext(nc) as tc:
        with tc.tile_pool(name="sb", bufs=1) as sb:
            vAT_sb0 = sb.tile([P, 8, RANK], BF16, tag="vAT")
            nc.gpsimd.dma_start(out=vAT_sb0, in_=vAT.rearrange("(co ci) r -> ci co r", ci=P))
            xT_sb = sb.tile([P, 8, T], BF16, tag="xT")
            xTr = xT.rearrange("(co ci) t -> ci co t", ci=P)
            for cc in range(8):
                nc.gpsimd.dma_start(out=xT_sb[:, cc, :], in_=xTr[:, cc, :])
            qAT_sb = sb.tile([P, 8, RANK], BF16, tag="qAT")
            nc.gpsimd.dma_start(out=qAT_sb, in_=qAT.rearrange("(co ci) r -> ci co r", ci=P))
            kAT_sb = sb.tile([P, 8, RANK], BF16, tag="kAT")
            nc.gpsimd.dma_start(out=kAT_sb, in_=kAT.rearrange("(co ci) r -> ci co r", ci=P))
            vAT_sb = vAT_sb0
            qBT_sb = sb.tile([RANK, G], BF16, tag="qBT")
            nc.gpsimd.dma_start(out=qBT_sb, in_=qBT[:, :])
            kBT_sb = sb.tile([RANK, G], BF16, tag="kBT")
            nc.gpsimd.dma_start(out=kBT_sb, in_=kBT[:, :])
            vBT_sb = sb.tile([RANK, G], BF16, tag="vBT")
            nc.gpsimd.dma_start(out=vBT_sb, in_=vBT[:, :])
            cAT2_sb = sb.tile([P, NPAIR, RANK], BF16, tag="cAT")
            nc.gpsimd.dma_start(out=cAT2_sb, in_=cAT.rearrange("(hp p) r -> p hp r", p=P))
            cBT_sb = sb.tile([RANK, C], BF16, tag="cBT")
            nc.gpsimd.dma_start(out=cBT_sb, in_=cBT[:, :])
            mask_sb = sb.tile([P, P], BF16, tag="mask")
            nc.gpsimd.dma_start(out=mask_sb, in_=maskp[:, :])

            QT2 = sb.tile([P, NPAIR, T], BF16, tag="QT2")
            KT2 = sb.tile([P, NPAIR, T], BF16, tag="KT2")
            YT2 = sb.tile([P, NPAIR, T], BF16, tag="YT2")
            V_sb = sb.tile([P, 16, HPC, P], BF16, tag="Vsb")
            tTq = sb.tile([P, T], BF16, tag="tTq")
            tTk = sb.tile([P, T], BF16, tag="tTk")
            tTv = sb.tile([P, T], BF16, tag="tTv")
            rT_sb = sb.tile([P, T], BF16, tag="rT")

            nc.gpsimd.memset(V_sb[:, :, :, 64:P], 1.0)
            rstg = sb.tile([P, 512], F32, tag="rstg")
            nc.gpsimd.memset(rstg, 1.0)
            rcp_sb = sb.tile([P, 512], F32, tag="rcp")

            # ---- phase A: t^T = A @ x^T for q,k,v ----
            with (
                tc.tile_pool(name="psA", bufs=2, space="PSUM") as psA,
                tc.tile_pool(name="psB", bufs=2, space="PSUM") as psB,
                tc.tile_pool(name="psV", bufs=2, space="PSUM") as psV,
            ):
                for pi, (AT_sb, tT) in enumerate(
                    ((vAT_sb, tTv), (qAT_sb, tTq), (kAT_sb, tTk))
                ):
                    for th in range(2):
                        h0 = th * 1024
                        pt = psA.tile([P, 1024], F32, tag="psA",
                                      name=f"psA{pi}_{th}")
                        for cc in range(8):
                            for t0, tw in _chunks(1024, 512):
                                nc.tensor.matmul(
                                    pt[:, t0:t0 + tw],
                                    AT_sb[:, cc, :],
                                    xT_sb[:, cc, h0 + t0:h0 + t0 + tw],
                                    start=(cc == 0), stop=(cc == 7),
                                )
                        nc.vector.tensor_copy(out=tT[:, h0:h0 + 1024], in_=pt)

                # ---- phase B: V keys-major with ones column ----
                for ti in range(16):
                    pv = psV.tile([P, G], F32, tag="psV")
                    nc.tensor.matmul(
                        pv, tTv[:, ti * 128:(ti + 1) * 128], vBT_sb,
                        start=True, stop=True,
                    )
                    nc.vector.tensor_copy(
                        out=V_sb[:, ti, :, 0:64],
                        in_=pv.rearrange("p (h d) -> p h d", d=64),
                    )

                # ---- phase B: pair-stacked Q^T, K^T  (M=128 = 2 heads) ----
                for BT_sb, dest, tT in ((qBT_sb, QT2, tTq), (kBT_sb, KT2, tTk)):
                    for hp in range(NPAIR):
                        for t0, tw in _chunks(T, 512):
                            p2 = psB.tile([P, 512], F32, tag="psB")
                            nc.tensor.matmul(
                                p2[:, :tw],
                                BT_sb[:, hp * P:(hp + 1) * P],
                                tT[:, t0:t0 + tw],
                                start=True, stop=True,
                            )
                            nc.vector.tensor_copy(
                                out=dest[:, hp, t0:t0 + tw], in_=p2[:, :tw])

            # ---- attention: head pairs, 2 concurrent row-groups ----
            # Strip loop is software-pipelined: strip kb+1's score matmuls are
            # emitted BEFORE strip kb's PV matmuls so the PE FIFO always has
            # exp-independent work while ScalarE runs exp on the prior strip.
            with (
                tc.tile_pool(name="psS", bufs=2, space="PSUM") as psS,
                tc.tile_pool(name="psPV", bufs=4, space="PSUM") as psPV,
                tc.tile_pool(name="ptp", bufs=6) as ptp,
                tc.tile_pool(name="nrm", bufs=8) as nrm,
            ):
                for hp in range(NPAIR):
                    for hc in range(2):
                        # pvt[e][j2]: accumulators for head e of the pair,
                        # q-panel j = 2*hc + j2; row 64 = softmax denominator
                        pvt = [[psPV.tile([P, 512], F32, tag="pv",
                                          name=f"pv{hp}_{hc}_{e}_{j2}")
                                for j2 in range(2)] for e in range(2)]
                        nkb = 8 if hc == 0 else 16

                        def emit_scores(kb, e, sps):
                            qlo = max(1024 * hc, 128 * kb)
                            w = 1024 * (hc + 1) - qlo
                            for c0, cw in _chunks(w, 512):
                                nc.tensor.matmul(
                                    sps[:, c0:c0 + cw],
                                    KT2[64 * e:64 * e + 64, hp,
                                        kb * 128:(kb + 1) * 128],
                                    QT2[64 * e:64 * e + 64, hp,
                                        qlo + c0:qlo + c0 + cw],
                                    start=True, stop=True,
                                )

                        def emit_exp(kb, e, sps, ptile):
                            qlo = max(1024 * hc, 128 * kb)
                            w = 1024 * (hc + 1) - qlo
                            nc.scalar.activation(
                                ptile[:, :w], sps[:, :w], Exp, scale=0.125)
                            if kb // 8 == hc:
                                nc.vector.tensor_tensor(
                                    out=ptile[:, 0:P], in0=ptile[:, 0:P],
                                    in1=mask_sb, op=MUL)

                        def emit_pv(kb, e, ptile):
                            qlo = max(1024 * hc, 128 * kb)
                            for j2 in range(2):
                                j = 2 * hc + j2
                                r0 = 512 * j
                                if 128 * kb >= r0 + 512:
                                    continue
                                c0 = max(qlo, r0)
                                cw = r0 + 512 - c0
                                nc.tensor.matmul(
                                    pvt[e][j2][:, c0 - r0:c0 - r0 + cw],
                                    V_sb[:, kb, 2 * hp + e, :],
                                    ptile[:, c0 - qlo:c0 - qlo + cw],
                                    start=(kb == 0), stop=(kb == 4 * j + 3),
                                )
                            for j2 in range(2):
                                j = 2 * hc + j2
                                if kb == 4 * j + 3:
                                    r0 = 512 * j
                                    nc.vector.tensor_copy(
                                        out=rstg[0:64, :],
                                        in_=pvt[e][j2][64:P, :])
                                    nc.vector.reciprocal_approx_fast(
                                        out=rcp_sb, in_=rstg)
                                    nc.vector.tensor_tensor(
                                        out=YT2[64 * e:64 * e + 64, hp,
                                                r0:r0 + 512],
                                        in0=pvt[e][j2][0:64, :],
                                        in1=rcp_sb[0:64, :], op=MUL)

                        # software pipeline, interleaved per head lane so every
                        # PE instruction is ready when the FIFO reaches it:
                        #   sc_e(kb) | pv_e(kb-1) | sc_o(kb) | pv_o(kb-1)
                        prev = None
                        for kb in range(nkb):
                            sps = [psS.tile([P, 1024], F32, tag="s",
                                            name=f"s{hp}_{hc}_{kb}_{e}")
                                   for e in range(2)]
                            ptile = [ptp.tile([P, 1024], BF16, tag="pt",
                                              name=f"p{hp}_{hc}_{kb}_{e}")
                                     for e in range(2)]
                            for e in range(2):
                                emit_scores(kb, e, sps[e])
                                emit_exp(kb, e, sps[e], ptile[e])
                                if prev is not None:
                                    emit_pv(prev[0], e, prev[1][e])
                            prev = (kb, ptile)
                        for e in range(2):
                            emit_pv(prev[0], e, prev[1][e])

            # ---- phase D: c_proj ----
            # stage 1 gets its own pool block so its 4 PSUM banks are freed
            # before the output loop, letting psO run 6 banks deep.
            with tc.tile_pool(name="psD", bufs=1, space="PSUM") as psD:
                pr = psD.tile([P, T], F32, tag="r")
                for hp in range(NPAIR):
                    for t0, tw in _chunks(T, 512):
                        nc.tensor.matmul(
                            pr[:, t0:t0 + tw], cAT2_sb[:, hp, :],
                            YT2[:, hp, t0:t0 + tw],
                            start=(hp == 0), stop=(hp == NPAIR - 1),
                        )
                for t0, tw in _chunks(T, 512):
                    nc.vector.tensor_copy(
                        out=rT_sb[:, t0:t0 + tw], in_=pr[:, t0:t0 + tw])
            with (
                tc.tile_pool(name="psO", bufs=6, space="PSUM") as psO,
                tc.tile_pool(name="ost", bufs=6) as ost,
            ):
                for ti in range(16):
                    for nn in range(2):
                        po = psO.tile([P, 512], F32, tag="o")
                        nc.tensor.matmul(
                            po, rT_sb[:, ti * 128:(ti + 1) * 128],
                            cBT_sb[:, nn * 512:(nn + 1) * 512],
                            start=True, stop=True,
                        )
                        ob = ost.tile([P, 512], BF16, tag="ob")
                        nc.vector.tensor_copy(out=ob, in_=po)
                        nc.sync.dma_start(
                            out=out[ti * 128:(ti + 1) * 128,
                                    nn * 512:(nn + 1) * 512],
                            in_=ob,
                        )
    nc.finalize()
    return nc


def make_in_maps(x, qA, qB, kA, kB, vA, vB, cA, cB):
    x, qA, qB, kA, kB, vA, vB, cA, cB = [
        np.asarray(a, dtype=np.float32) for a in (x, qA, qB, kA, kB, vA, vB, cA, cB)
    ]
    mask01 = np.where(
        np.arange(P)[:, None] <= np.arange(P)[None, :], 1.0, 0.0
    ).astype(BF)
    qATn = np.ascontiguousarray(qA.T).astype(BF)
    kATn = np.ascontiguousarray(kA.T).astype(BF)
    vATn = np.ascontiguousarray(vA.T).astype(BF)
    cBTn = np.ascontiguousarray(cB.T).astype(BF)
    in_maps = []
    for c in range(NCORES):
        b, g = divmod(c, 2)
        sl = slice(g * G, (g + 1) * G)
        in_maps.append({
            "xT": np.ascontiguousarray(x[b].T).astype(BF),
            "qAT": qATn, "kAT": kATn, "vAT": vATn,
            "qBT": np.ascontiguousarray(qB[sl, :].T).astype(BF),
            "kBT": np.ascontiguousarray(kB[sl, :].T).astype(BF),
            "vBT": np.ascontiguousarray(vB[sl, :].T).astype(BF),
            "cAT": np.ascontiguousarray(cA[:, sl].T).astype(BF),
            "cBT": cBTn,
            "mask01": mask01,
        })
    return in_maps


def combine(parts):
    return np.stack(
        [parts[2 * b].astype(np.float32) + parts[2 * b + 1].astype(np.float32)
         for b in range(B)], axis=0)


def kernel(x, qA, qB, kA, kB, vA, vB, cA, cB):
    global _NC_CACHE
    if _NC_CACHE is None:
        _NC_CACHE = build()
    in_maps = make_in_maps(x, qA, qB, kA, kB, vA, vB, cA, cB)
    res = run_bass_kernel_spmd(_NC_CACHE, in_maps, list(range(NCORES))).results
    return combine([res[c]["out"] for c in range(NCORES)])
